# revision 12
# baseline (speedup 1.0000x reference)
"""Trainium2 Bass kernel for DenseConv2d.

Conv2d: input (32,128,56,56) f32, weight (256,128,3,3) f32, bias (256,) f32,
stride 1, pad 1, dilation 1 -> output (32,256,56,56) f32.

Strategy: data-parallel over batch across 8 NeuronCores (4 images per core).
Per core the conv is computed as accumulated matmuls (one per kernel tap)
into PSUM: out[co, pix] += W[kh,kw][ci,co].T @ x_pad[ci, shifted pix window].
Operands are cast host-side to bfloat16 so the PE array streams 1 column per
cycle (448-col matmul every ~190 ns warm) and weight loads ride the FWL fast
path; accumulation stays fp32 in PSUM.

Output rows are processed in groups per (img, cot): rows 1-8, 9-16, ...,
41-48 (8 rows, all 9 taps), rows 49-54 (6 rows), row 55 (kh=0,1 only) and
row 0 (kh=1,2 only) - the single-row edge groups skip the taps that would
only multiply the zero padding, and row 0 is processed LAST so the final
drain (6 tiny matmuls + a 1-row store) is short. Rows 49-55 share one
7-row store.

Input arrives as 10-row chunks (img 0, one per row-group, first one split
in two) interleaved across both HWDGE queues so the first group's data
lands as early as possible; weights for cot0 are split in three tap-triples
across the queues. The bias rides the gpsimd SWDGE ring to stay off the
critical queues. Images 1-3 use bigger 18-row chunks. Warmup matmuls on
scratch data bridge the DMA wait so the PE HAM clock-gate is at 2.4 GHz
when real work arrives. Layout prep (padding, channel-major transpose,
bf16 cast) is host-side numpy.
"""

import sys

if "/opt/trn_rl_repo" not in sys.path:
    sys.path.insert(0, "/opt/trn_rl_repo")

import numpy as np

N_CORES = 8
N, CI, H, W = 32, 128, 56, 56
CO, KH, KW = 256, 3, 3
NP_CORE = N // N_CORES          # images per core
HP, WP = H + 2, W + 2           # padded spatial dims
COT = CO // 128                 # out-channel tiles of 128
RB = 8                          # output rows per full matmul block
N_WARM_BIG = 28                 # 128-col warmup matmuls (~107 ns cold each)
N_WARM_TINY = 8                 # 64-col warmup matmuls (~53 ns cold each)

# Row groups per (img, cot): (out_r0, nrows, tap positions). Taps reading
# only zero padding (kh=0 at row 0, kh=2 at row 55) are skipped. Row 0 goes
# last so the final group of the whole kernel is tiny.
ALLT = list(range(9))
# Group 1's taps run in DMA-arrival order (w taps 0-2 land first, then
# 6-8, then 3-5) - accumulation order is numerically irrelevant.
GROUPS = [
    (1, 8, [0, 1, 2, 6, 7, 8, 3, 4, 5]),
    (9, 8, ALLT), (17, 8, ALLT), (25, 8, ALLT),
    (33, 8, ALLT), (41, 8, ALLT), (49, 6, ALLT),
    (55, 1, [0, 1, 2, 3, 4, 5]),     # kh = 0,1
    (0, 1, [3, 4, 5, 6, 7, 8]),      # kh = 1,2
]
# img0 chunks: (padded base row, nrows); chunk k serves group k (and chunk 0
# also group 8 / chunk 6 also group 7).
CH0 = [(8 * k + 1, 10) for k in range(6)] + [(49, 8)]
G2C0 = [0, 1, 2, 3, 4, 5, 6, 6, 0]
# img 1-3 chunks (bigger):
CHI = [(1, 18), (17, 18), (33, 18), (49, 8)]
G2CI = [0, 0, 1, 1, 2, 2, 3, 3, 0]

_CACHE = {}


def _build_program():
    import concourse.mybir as mybir
    from concourse import bacc
    from concourse.tile import TileContext

    nc = bacc.Bacc(None, target_bir_lowering=False)

    bf16 = mybir.dt.bfloat16
    f32 = mybir.dt.float32

    x_d = nc.dram_tensor("x", [CI, NP_CORE, HP, WP], bf16,
                         kind="ExternalInput")
    w_d = nc.dram_tensor("w", [CI, COT, KH * KW, 128], bf16,
                         kind="ExternalInput")
    b_d = nc.dram_tensor("b2", [128, COT], f32,
                         kind="ExternalInput")
    y_d = nc.dram_tensor("y", [COT, 128, NP_CORE, H, W], f32,
                         kind="ExternalOutput")

    with TileContext(nc) as tc:
        with (
            tc.tile_pool(name="xin", bufs=1) as xpool,
            tc.tile_pool(name="wpool", bufs=1) as wpool,
            tc.tile_pool(name="bpool", bufs=1) as bpool,
            tc.tile_pool(name="psum", bufs=8, space="PSUM") as ppool,
            tc.tile_pool(name="out", bufs=6) as opool,
            tc.tile_pool(name="out7", bufs=2) as o7pool,
            tc.tile_pool(name="out1", bufs=2) as o1pool,
        ):
            # PE warmup on scratch data, concurrent with the input DMAs,
            # so the HAM clock-gate is at 2.4 GHz when real matmuls start.
            # Small scratch so the memset is short and the PE busy period
            # (which arms the HAM clock-gate) starts as early as possible.
            scratch = xpool.tile([CI, 128], bf16, tag="scratch")
            nc.gpsimd.memset(scratch, 0.0)
            wups = ppool.tile([128, RB * W], mybir.dt.float32, tag="ps")
            for _ in range(N_WARM_BIG):
                nc.tensor.matmul(wups[:, 0:128], scratch, scratch,
                                 start=True, stop=True)
            for _ in range(N_WARM_TINY):
                nc.tensor.matmul(wups[:, 0:64], scratch,
                                 scratch[:, 0:64], start=True, stop=True)

            wt = []
            for cot in range(COT):
                wtile = wpool.tile([CI, KH * KW, 128], bf16, tag=f"w{cot}")
                wt.append(wtile)
            bt = bpool.tile([128, COT], f32)

            xt = {}      # (img, chunk_idx) -> (tile, padded_base_row)

            def x_chunk_dma(img, ck, eng):
                base, rows = (CH0 if img == 0 else CHI)[ck]
                t = xpool.tile([CI, rows, WP], bf16, tag=f"x{img}_{ck}")
                eng.dma_start(out=t, in_=x_d[:, img, base:base + rows, :])
                xt[img, ck] = (t, base)

            # Critical path: the first group (img0, cot0, rows 1-8) needs
            # x rows 1-10 plus all 9 cot0 taps. Split those finely across
            # both HWDGE queues; the bias rides the gpsimd SWDGE ring.
            nc.gpsimd.dma_start(out=bt, in_=b_d[:, :])
            # x0 chunk0 in two pieces so taps kh=0 can start on rows 1-8.
            t0 = xpool.tile([CI, 10, WP], bf16, tag="x0_0")
            nc.sync.dma_start(out=t0[:, 0:8, :], in_=x_d[:, 0, 1:9, :])
            xt[0, 0] = (t0, 1)
            nc.scalar.dma_start(out=wt[0][:, 0:3, :], in_=w_d[:, 0, 0:3, :])
            # rows 9-10 and the pos-3..5 taps ride the gpsimd ring: off the
            # critical HWDGE queues (group 1 runs those taps LAST, so even
            # the slower SWDGE path lands them in time), which lets chunk 1
            # start ~0.8 us earlier on the scalar ring.
            nc.gpsimd.dma_start(out=t0[:, 8:10, :], in_=x_d[:, 0, 9:11, :])
            nc.gpsimd.dma_start(out=wt[0][:, 3:6, :], in_=w_d[:, 0, 3:6, :])
            nc.sync.dma_start(out=wt[0][:, 6:9, :], in_=w_d[:, 0, 6:9, :])
            # img0 chunks alternate queues so neither ring backs up; w1 and
            # the img1-3 chunks follow on the scalar ring (needed late).
            x_chunk_dma(0, 1, nc.scalar)
            x_chunk_dma(0, 2, nc.sync)
            x_chunk_dma(0, 3, nc.scalar)
            x_chunk_dma(0, 4, nc.sync)
            x_chunk_dma(0, 5, nc.scalar)
            x_chunk_dma(0, 6, nc.sync)
            nc.scalar.dma_start(out=wt[1], in_=w_d[:, 1, :, :])
            for img in range(1, NP_CORE):
                for ck in range(len(CHI)):
                    x_chunk_dma(img, ck, nc.scalar)

            for img in range(NP_CORE):
                g2c = G2C0 if img == 0 else G2CI
                for cot in range(COT):
                    ot7 = None
                    for gi, (r0, nr, taps) in enumerate(GROUPS):
                        ps = ppool.tile([128, RB, W], mybir.dt.float32,
                                        tag="ps")
                        xc, base = xt[img, g2c[gi]]
                        for ti, pos in enumerate(taps):
                            kh, kw = divmod(pos, KW)
                            lo = r0 + kh - base
                            rhs = xc[:, lo:lo + nr, kw:kw + W]
                            nc.tensor.matmul(
                                ps[:, 0:nr, :], wt[cot][:, pos, :], rhs,
                                start=(ti == 0), stop=(ti == len(taps) - 1),
                            )
                        if nr == RB:
                            ot = opool.tile([128, RB, W], f32)
                            nc.vector.tensor_scalar_add(
                                ot, ps, bt[:, cot:cot + 1])
                            nc.sync.dma_start(
                                out=y_d[cot, :, img, r0:r0 + RB, :], in_=ot)
                        elif nr == 6:
                            # rows 49-54: copy into the shared 7-row tile;
                            # stored together with row 55.
                            ot7 = o7pool.tile([128, 7, W], f32, tag="ot7")
                            nc.vector.tensor_scalar_add(
                                ot7[:, 0:6, :], ps[:, 0:6, :],
                                bt[:, cot:cot + 1])
                        elif r0 == H - 1:
                            # row 55 copy rides the scalar engine so it does
                            # not queue behind row-49-54's copy on vector.
                            nc.scalar.activation(
                                ot7[:, 6:7, :], ps[:, 0:1, :],
                                mybir.ActivationFunctionType.Identity,
                                bias=bt[:, cot:cot + 1])
                            nc.sync.dma_start(
                                out=y_d[cot, :, img, 49:56, :], in_=ot7)
                        else:
                            # row 0 - the last, tiny group. Its store rides
                            # the scalar ring (drained by then) so it does
                            # not queue behind the 7-row store's issue.
                            ot1 = o1pool.tile([128, 1, W], f32, tag="ot1")
                            nc.scalar.activation(
                                ot1, ps[:, 0:1, :],
                                mybir.ActivationFunctionType.Identity,
                                bias=bt[:, cot:cot + 1])
                            # Final store split by partition halves across
                            # both rings: half the descriptors per ring.
                            nc.scalar.dma_start(
                                out=y_d[cot, 0:64, img, 0:1, :],
                                in_=ot1[0:64, :, :])
                            nc.sync.dma_start(
                                out=y_d[cot, 64:128, img, 0:1, :],
                                in_=ot1[64:128, :, :])

    nc.compile()
    return nc


def prep_in_maps(input, weight, bias):
    """Host-side layout prep -> one in_map per core."""
    import ml_dtypes

    bf16 = ml_dtypes.bfloat16
    xp = np.pad(input, ((0, 0), (0, 0), (1, 1), (1, 1))).astype(bf16)
    # weight [co, ci, kh, kw] -> [ci, cot, (kh kw), cop]
    wr = np.ascontiguousarray(
        weight.transpose(1, 2, 3, 0).reshape(CI, KH * KW, COT, 128)
        .transpose(0, 2, 1, 3)).astype(bf16)
    b2 = np.ascontiguousarray(bias.reshape(COT, 128).T)

    in_maps = []
    for c in range(N_CORES):
        xc = np.ascontiguousarray(
            xp[c * NP_CORE:(c + 1) * NP_CORE].transpose(1, 0, 2, 3))
        in_maps.append({"x": xc, "w": wr, "b2": b2})
    return in_maps


def kernel(input, weight, bias):
    input = np.asarray(input, dtype=np.float32)
    weight = np.asarray(weight, dtype=np.float32)
    bias = np.asarray(bias, dtype=np.float32)

    if "nc" not in _CACHE:
        _CACHE["nc"] = _build_program()
    nc = _CACHE["nc"]

    from concourse.bass_utils import run_bass_kernel_spmd

    in_maps = prep_in_maps(input, weight, bias)
    res = run_bass_kernel_spmd(nc, in_maps, core_ids=list(range(N_CORES)))

    out = np.empty((N, CO, H, W), dtype=np.float32)
    for c in range(N_CORES):
        y = np.asarray(res.results[c]["y"], dtype=np.float32)  # [COT,128,NP,H,W]
        out[c * NP_CORE:(c + 1) * NP_CORE] = (
            y.transpose(2, 0, 1, 3, 4).reshape(NP_CORE, CO, H, W))
    return out
